# revision 1
# baseline (speedup 1.0000x reference)
"""TRN2 Bass kernel for nn_AttributeClassifierHeaders (dense per-head MLP).

Computes y[b, a] = sigmoid(gelu(x @ W1[a] + b1[a]) . W2[a] + b2[a]) for 40
heads, sharded 5 heads per NeuronCore across 8 cores (head-parallel: each
head's weights are independent; x is replicated).

Formulation per core (transposed): for each head a and hid-tile m,
  hT[m] = gelu(W1[a,:,m128].T @ x.T + b1) as [128 hid, 512 batch] tiles,
computed on the PE with float32r operands (full bf16-rate at N=512, ~1e-4
rel err vs the fp32 reference -- measured 6.5e-5 end to end), gelu+bias
fused on ScalarE out of PSUM, then the per-head dot product accumulates
over m as M=1 matmuls into a second PSUM bank (emitted one stage-1 group
late so the in-order PE queue never waits on ACT). x.T is resident in SBUF
as batch halves; W1 streams from HBM (packed host-side for contiguous
per-(a,m) 1 MiB DMAs). Sigmoid+b2 run once at the end (one extra
activation-table load).
"""
import os
import sys
from contextlib import ExitStack

import numpy as np

for _p in ("/root/.axon_site/_ro/trn_rl_repo", "/opt/trn_rl_repo"):
    if os.path.isdir(_p) and _p not in sys.path:
        sys.path.append(_p)

import jax  # noqa: E402
from jax.sharding import Mesh, PartitionSpec, NamedSharding  # noqa: E402
from jax.experimental.shard_map import shard_map  # noqa: E402

import concourse.bacc as bacc  # noqa: E402
import concourse.tile as tile  # noqa: E402
from concourse import mybir, bass2jax  # noqa: E402

F32 = mybir.dt.float32
F32R = mybir.dt.float32r
AF = mybir.ActivationFunctionType

# problem shape (hardcoded; see module docstring)
B, D, A, H = 4096, 2048, 40, 1024
NCORES = 8
APC = A // NCORES        # 5 heads per core
KT = D // 128            # 16 contraction tiles
MT = H // 128            # 8 hid tiles
NQ = 4                   # batch blocks (4 => double-buffered quarters; W1 streamed 4x,
                         # ~44% DMA duty, but batch-block reloads fully overlap compute)
QTR = B // NQ            # resident batch block
NCH = QTR // 512         # 512-wide chunks per block


def build_program(repeat: int = 0, nq: int = NQ):
    qtr = B // nq
    nch = qtr // 512
    xbufs = 2 if nq >= 4 else 1
    nc = bacc.Bacc("TRN2", target_bir_lowering=False, debug=False)
    xT_d = nc.dram_tensor("xT", [D, B], F32R, kind="ExternalInput").ap()
    w1_d = nc.dram_tensor("w1p", [APC, MT, 128, KT * 128], F32R, kind="ExternalInput").ap()
    b1_d = nc.dram_tensor("b1p", [APC, 128, MT], F32, kind="ExternalInput").ap()
    w2_d = nc.dram_tensor("w2p", [APC, 128, MT], F32R, kind="ExternalInput").ap()
    b2_d = nc.dram_tensor("b2p", [APC, 1], F32, kind="ExternalInput").ap()
    y_d = nc.dram_tensor("y", [APC, B], F32, kind="ExternalOutput").ap()

    with tile.TileContext(nc) as tc, ExitStack() as ctx:
        const = ctx.enter_context(tc.tile_pool(name="const", bufs=1))
        xp = ctx.enter_context(tc.tile_pool(name="xp", bufs=xbufs))
        wp = ctx.enter_context(tc.tile_pool(name="wp", bufs=2))
        sp = ctx.enter_context(tc.tile_pool(name="sp", bufs=3))
        hp = ctx.enter_context(tc.tile_pool(name="hp", bufs=5))
        lg = ctx.enter_context(tc.tile_pool(name="lg", bufs=1))
        ps1 = ctx.enter_context(tc.tile_pool(name="ps1", bufs=4, space="PSUM"))
        ps2 = ctx.enter_context(tc.tile_pool(name="ps2", bufs=4, space="PSUM"))

        def body():
            b1t = const.tile([128, APC * MT], F32, tag="b1t")
            w2t = const.tile([128, APC * MT], F32R, tag="w2t")
            b2t = const.tile([APC, 1], F32, tag="b2t")
            for a in range(APC):
                nc.sync.dma_start(b1t[:, a * MT:(a + 1) * MT], b1_d[a])
                nc.sync.dma_start(w2t[:, a * MT:(a + 1) * MT], w2_d[a])
            nc.sync.dma_start(b2t[:], b2_d[:])
            logits = lg.tile([APC, B], F32, tag="logits")
            for hf in range(nq):
                xq = []
                for k in range(KT):
                    t = xp.tile([128, qtr], F32R, tag=f"xq{k}")
                    nc.sync.dma_start(t[:], xT_d[k * 128:(k + 1) * 128,
                                                 hf * qtr:(hf + 1) * qtr])
                    xq.append(t)
                for a in range(APC):
                    psy = [None] * nch
                    # stage-2 matmuls are emitted one stage-1 group late so
                    # the in-order PE queue never waits on the gelu (ACT)
                    # that produces their rhs.
                    pending = []
                    for m in range(MT):
                        w = wp.tile([128, KT * 128], F32R, tag="w")
                        nc.sync.dma_start(w[:], w1_d[a, m])
                        # Boundary iterations run k-outermost so xq k-tiles
                        # are first-needed / last-read staggered by k: the
                        # next half's 16 MiB xq reload then overlaps compute
                        # instead of stalling the PE at the half boundary.
                        kouter = (m == 0 and a == 0) or \
                                 (m == MT - 1 and a == APC - 1)
                        if kouter:
                            pts = []
                            for n in range(nch):
                                pt_n = ps1.tile([128, 512], F32, tag="ps1",
                                                name=f"pt{n}")
                                pts.append(pt_n)
                            for k in range(KT):
                                for n in range(nch):
                                    nc.tensor.matmul(
                                        pts[n][:],
                                        w[:, k * 128:(k + 1) * 128],
                                        xq[k][:, n * 512:(n + 1) * 512],
                                        start=(k == 0), stop=(k == KT - 1))
                            while pending:
                                pending.pop(0)()
                        def tail(n, pt, m=m):
                            ht = hp.tile([128, 512], F32R, tag="ht",
                                         name="ht")
                            nc.scalar.activation(
                                ht[:], pt[:], AF.Gelu,
                                bias=b1t[:, a * MT + m:a * MT + m + 1])
                            if m == 0:
                                psy_t = ps2.tile([1, 512], F32, tag="psy",
                                                 name="psy_t")
                                psy[n] = psy_t

                            def emit_stage2(m=m, n=n, ht=ht):
                                nc.tensor.matmul(
                                    psy[n][:],
                                    w2t[:, a * MT + m:a * MT + m + 1],
                                    ht[:],
                                    start=(m == 0), stop=(m == MT - 1),
                                    skip_group_check=True)
                            pending.append(emit_stage2)

                        if kouter:
                            for n in range(nch):
                                tail(n, pts[n])
                        else:
                            for n in range(nch):
                                pt = ps1.tile([128, 512], F32, tag="ps1")
                                for k in range(KT):
                                    nc.tensor.matmul(
                                        pt[:],
                                        w[:, k * 128:(k + 1) * 128],
                                        xq[k][:, n * 512:(n + 1) * 512],
                                        start=(k == 0), stop=(k == KT - 1))
                                if pending:
                                    pending.pop(0)()
                                tail(n, pt)
                    while pending:
                        pending.pop(0)()
                    for n in range(nch):
                        stg = sp.tile([1, 512], F32, tag="stg")
                        nc.vector.tensor_copy(stg[:], psy[n][:])
                        nc.sync.dma_start(
                            logits[a:a + 1,
                                   hf * qtr + n * 512:hf * qtr + (n + 1) * 512],
                            stg[:])
            yt = lg.tile([APC, B], F32, tag="yt")
            nc.scalar.activation(yt[:], logits[:], AF.Sigmoid, bias=b2t[:])
            nc.sync.dma_start(y_d[:], yt[:])

        if repeat and repeat > 1:
            with tc.For_i(0, repeat, 1):
                body()
        else:
            body()
    nc.compile()
    return nc


class _Runner:
    """jit-once PJRT runner for a prebuilt Bass program (8-core SPMD)."""

    def __init__(self, nc, n_cores):
        bass2jax.install_neuronx_cc_hook()
        self.nc = nc
        self.n_cores = n_cores
        in_names, out_names, out_avals, zero_outs = [], [], [], []
        for alloc in nc.m.functions[0].allocations:
            if not isinstance(alloc, mybir.MemoryLocationSet):
                continue
            name = alloc.memorylocations[0].name
            if alloc.kind == "ExternalInput":
                in_names.append(name)
            elif alloc.kind == "ExternalOutput":
                shape = tuple(alloc.tensor_shape)
                dtype = mybir.dt.np(alloc.dtype)
                out_names.append(name)
                out_avals.append(jax.core.ShapedArray(shape, dtype))
                zero_outs.append(np.zeros(shape, dtype))
        partition_name = (nc.partition_id_tensor.name
                          if nc.partition_id_tensor else None)
        if partition_name is not None and partition_name in in_names:
            in_names.remove(partition_name)
        self.in_names = in_names
        self.out_names = out_names
        self.zero_outs = zero_outs
        n_params = len(in_names)
        n_outs = len(out_avals)
        all_in_names = list(in_names) + list(out_names)
        if partition_name is not None:
            all_in_names.append(partition_name)
        donate = tuple(range(n_params, n_params + n_outs))

        def _body(*args):
            operands = list(args)
            if partition_name is not None:
                operands.append(bass2jax.partition_id_tensor())
            outs = bass2jax._bass_exec_p.bind(
                *operands,
                out_avals=tuple(out_avals),
                in_names=tuple(all_in_names),
                out_names=tuple(out_names),
                lowering_input_output_aliases=(),
                sim_require_finite=True,
                sim_require_nnan=True,
                nc=nc,
            )
            return tuple(outs)

        devices = jax.devices()[:n_cores]
        assert len(devices) == n_cores, f"need {n_cores} neuron cores"
        self.mesh = Mesh(np.asarray(devices), ("core",))
        in_specs = (PartitionSpec("core"),) * (n_params + n_outs)
        out_specs = (PartitionSpec("core"),) * n_outs
        self.fn = jax.jit(
            shard_map(_body, mesh=self.mesh, in_specs=in_specs,
                      out_specs=out_specs, check_rep=False),
            donate_argnums=donate, keep_unused=True,
        )
        self._dev_inputs = None

    def put_inputs(self, in_maps):
        sharding = NamedSharding(self.mesh, PartitionSpec("core"))
        self._dev_inputs = [
            jax.device_put(
                np.concatenate([np.asarray(m[name]) for m in in_maps], axis=0),
                sharding)
            for name in self.in_names
        ]

    def run(self):
        sharding = NamedSharding(self.mesh, PartitionSpec("core"))
        zouts = [jax.device_put(np.concatenate([z] * self.n_cores, axis=0),
                                sharding) for z in self.zero_outs]
        outs = self.fn(*self._dev_inputs, *zouts)
        jax.block_until_ready(outs)
        return outs

    def run_np(self):
        outs = self.run()
        res = []
        for c in range(self.n_cores):
            d = {}
            for i, name in enumerate(self.out_names):
                full = np.asarray(outs[i])
                per = full.shape[0] // self.n_cores
                d[name] = full[c * per:(c + 1) * per]
            res.append(d)
        return res


_CACHE = {}


def _get_runner(repeat=0):
    if repeat not in _CACHE:
        _CACHE[repeat] = _Runner(build_program(repeat), NCORES)
    return _CACHE[repeat]


def make_in_maps(x, W1, b1, W2, b2):
    x = np.asarray(x, dtype=np.float32)
    W1 = np.asarray(W1, dtype=np.float32)
    b1 = np.asarray(b1, dtype=np.float32)
    W2 = np.asarray(W2, dtype=np.float32)
    b2 = np.asarray(b2, dtype=np.float32)
    xT = np.ascontiguousarray(x.T)
    # W1p[a, m, p, k*128+c] = W1[a, k*128+p, m*128+c]  (per-(a,m) contiguous
    # 1 MiB block whose partition rows are 8 KiB contiguous runs)
    W1p = np.ascontiguousarray(
        W1.reshape(A, KT, 128, MT, 128).transpose(0, 3, 2, 1, 4)
        .reshape(A, MT, 128, KT * 128))
    b1p = np.ascontiguousarray(b1.reshape(A, MT, 128).transpose(0, 2, 1))
    W2p = np.ascontiguousarray(W2.reshape(A, MT, 128).transpose(0, 2, 1))
    b2p = np.ascontiguousarray(b2.reshape(A, 1))
    in_maps = []
    for c in range(NCORES):
        s = slice(c * APC, (c + 1) * APC)
        in_maps.append({"xT": xT, "w1p": W1p[s], "b1p": b1p[s],
                        "w2p": W2p[s], "b2p": b2p[s]})
    return in_maps


def kernel(x, W1, b1, W2, b2):
    in_maps = make_in_maps(x, W1, b1, W2, b2)
    r = _get_runner(0)
    r.put_inputs(in_maps)
    outs = r.run_np()
    y = np.concatenate([outs[c]["y"] for c in range(NCORES)], axis=0)
    return np.ascontiguousarray(y.T).astype(np.float32)



# revision 2
# speedup vs baseline: 2.0546x; 2.0546x over previous
"""TRN2 Bass kernel for nn_AttributeClassifierHeaders (dense per-head MLP).

Computes y[b, a] = sigmoid(gelu(x @ W1[a] + b1[a]) . W2[a] + b2[a]) for 40
heads, sharded 5 heads per NeuronCore across 8 cores (head-parallel: each
head's weights are independent; x is replicated).

Stage-1 runs on the PE in fp8(e4m3) with perf_mode=DoubleRow: x and W1 are
quantized host-side (scales SX/SW keep values in e4m3's normal range; the
gelu activation un-scales via its fused `scale`), the contraction runs as 8
double-chunks of 256 (two fp8 weights per PE cell => 2x bf16 throughput).
End-to-end rel err vs the fp32 reference is ~1.3e-2 (CPU-simulated and
HW-verified), inside the 2e-2 gate. Layout per 256-chunk c: slot (p, i)
holds contraction index d = c*256 + i*128 + p, identically for the
stationary W1 tile [128, 2, 128] and the moving x tile [128, 2, 512].

gelu+bias+descale fuse on ScalarE out of PSUM (bf16 out); the per-head dot
product accumulates over hid-tiles m as M=1 bf16 matmuls into a second PSUM
bank, emitted one stage-1 group late so the in-order PE queue never waits
on ACT. x is SBUF-resident in fp8 as two batch halves (8 MiB total, both
live); W1 streams from HBM (packed host-side for contiguous per-(a,m)
512 KiB DMAs, streamed once per half). Sigmoid+b2 run once at the end.
"""
import os
import sys
from contextlib import ExitStack

import numpy as np
import ml_dtypes

for _p in ("/root/.axon_site/_ro/trn_rl_repo", "/opt/trn_rl_repo"):
    if os.path.isdir(_p) and _p not in sys.path:
        sys.path.append(_p)

import jax  # noqa: E402
from jax.sharding import Mesh, PartitionSpec, NamedSharding  # noqa: E402
from jax.experimental.shard_map import shard_map  # noqa: E402

import concourse.bacc as bacc  # noqa: E402
import concourse.tile as tile  # noqa: E402
from concourse import mybir, bass2jax  # noqa: E402

F32 = mybir.dt.float32
F8 = mybir.dt.float8e4
BF = mybir.dt.bfloat16
AF = mybir.ActivationFunctionType
DR = mybir.MatmulPerfMode.DoubleRow

# problem shape (hardcoded; see module docstring)
B, D, A, H = 4096, 2048, 40, 1024
NCORES = 8
APC = A // NCORES        # 5 heads per core
KT = D // 128            # 16 contraction 128-tiles
KC = KT // 2             # 8 DoubleRow 256-chunks
MT = H // 128            # 8 hid tiles
NQ = 2                   # batch halves (both resident in SBUF as fp8)
QTR = B // NQ            # 2048
NCH = QTR // 512         # 512-wide chunks per half

SX = 16.0                # x fp8 scale (|x|<~6 -> <96, e4m3 normal range)
SW = 4096.0              # W1 fp8 scale (|W1|<=0.0221 -> <=90.5)
INV = 1.0 / (SX * SW)    # descale fused into the gelu activation

E4NP = ml_dtypes.float8_e4m3   # == mybir.dt.np(float8e4): TRN variant, max 240
BFNP = ml_dtypes.bfloat16


def build_program(repeat: int = 0):
    nc = bacc.Bacc("TRN2", target_bir_lowering=False, debug=False)
    x8_d = nc.dram_tensor("x8", [NQ * KC, 128, 2 * QTR], F8,
                          kind="ExternalInput").ap()
    w1_d = nc.dram_tensor("w1p", [APC, MT, 128, KT, 128], F8,
                          kind="ExternalInput").ap()
    b1_d = nc.dram_tensor("b1p", [APC, 128, MT], F32, kind="ExternalInput").ap()
    w2_d = nc.dram_tensor("w2p", [APC, 128, MT], BF, kind="ExternalInput").ap()
    b2_d = nc.dram_tensor("b2p", [APC, 1], F32, kind="ExternalInput").ap()
    y_d = nc.dram_tensor("y", [APC, B], F32, kind="ExternalOutput").ap()

    with tile.TileContext(nc) as tc, ExitStack() as ctx:
        const = ctx.enter_context(tc.tile_pool(name="const", bufs=1))
        xp = ctx.enter_context(tc.tile_pool(name="xp", bufs=2))
        wp = ctx.enter_context(tc.tile_pool(name="wp", bufs=2))
        sp = ctx.enter_context(tc.tile_pool(name="sp", bufs=3))
        hp = ctx.enter_context(tc.tile_pool(name="hp", bufs=5))
        lg = ctx.enter_context(tc.tile_pool(name="lg", bufs=1))
        ps1 = ctx.enter_context(tc.tile_pool(name="ps1", bufs=4, space="PSUM"))
        ps2 = ctx.enter_context(tc.tile_pool(name="ps2", bufs=4, space="PSUM"))

        def body():
            b1t = const.tile([128, APC * MT], F32, tag="b1t")
            w2t = const.tile([128, APC * MT], BF, tag="w2t")
            b2t = const.tile([APC, 1], F32, tag="b2t")
            for a in range(APC):
                nc.sync.dma_start(b1t[:, a * MT:(a + 1) * MT], b1_d[a])
                nc.sync.dma_start(w2t[:, a * MT:(a + 1) * MT], w2_d[a])
            nc.sync.dma_start(b2t[:], b2_d[:])
            logits = lg.tile([APC, B], F32, tag="logits")
            # x chunk tiles for both halves, all live (8 MiB fp8 total).
            # Separate tiles per 256-chunk keep the DMA->matmul dependency
            # per-chunk so the first tile's c-outer loop starts as soon as
            # chunk 0 lands (instead of waiting for the full 8 MiB).
            xh = [[xp.tile([128, 2, QTR], F8, tag=f"xc{c}", name=f"x{hf}c{c}")
                   for c in range(KC)] for hf in range(NQ)]

            def dma_x(hf, c):
                nc.sync.dma_start(xh[hf][c][:], x8_d[hf * KC + c])

            dma_x(0, 0)
            first = True
            for hf in range(NQ):
                for a in range(APC):
                    psy = [None] * NCH
                    # stage-2 matmuls are emitted one stage-1 group late so
                    # the in-order PE queue never waits on the gelu (ACT)
                    # that produces their rhs.
                    pending = []
                    for m in range(MT):
                        w = wp.tile([128, KT, 128], F8, tag="w")
                        nc.sync.dma_start(w[:], w1_d[a, m])
                        if first:
                            # rest of half 0's x behind the first W1 tile,
                            # half 1's x behind that (all overlap compute)
                            for c in range(1, KC):
                                dma_x(0, c)
                            for c in range(KC):
                                dma_x(1, c)
                        kouter = first
                        first = False
                        if kouter:
                            # c-outermost so each x chunk is consumed as it
                            # arrives; uses NCH psum banks at once.
                            pts = []
                            for n in range(NCH):
                                pt_n = ps1.tile([128, 512], F32, tag="ps1",
                                                name=f"pt{n}")
                                pts.append(pt_n)
                            for c in range(KC):
                                for n in range(NCH):
                                    nc.tensor.matmul(
                                        pts[n][:],
                                        w[:, 2 * c:2 * c + 2, :],
                                        xh[hf][c][:, :, n * 512:(n + 1) * 512],
                                        start=(c == 0), stop=(c == KC - 1),
                                        perf_mode=DR)
                            while pending:
                                pending.pop(0)()

                        def tail(n, pt, m=m):
                            ht = hp.tile([128, 512], BF, tag="ht", name="ht")
                            nc.scalar.activation(
                                ht[:], pt[:], AF.Gelu,
                                bias=b1t[:, a * MT + m:a * MT + m + 1],
                                scale=INV)
                            if m == 0:
                                psy_t = ps2.tile([1, 512], F32, tag="psy",
                                                 name="psy_t")
                                psy[n] = psy_t

                            def emit_stage2(m=m, n=n, ht=ht):
                                nc.tensor.matmul(
                                    psy[n][:],
                                    w2t[:, a * MT + m:a * MT + m + 1],
                                    ht[:],
                                    start=(m == 0), stop=(m == MT - 1),
                                    skip_group_check=True)
                            pending.append(emit_stage2)

                        if kouter:
                            for n in range(NCH):
                                tail(n, pts[n])
                        else:
                            for n in range(NCH):
                                pt = ps1.tile([128, 512], F32, tag="ps1")
                                for c in range(KC):
                                    nc.tensor.matmul(
                                        pt[:],
                                        w[:, 2 * c:2 * c + 2, :],
                                        xh[hf][c][:, :, n * 512:(n + 1) * 512],
                                        start=(c == 0), stop=(c == KC - 1),
                                        perf_mode=DR)
                                if pending:
                                    pending.pop(0)()
                                tail(n, pt)
                    while pending:
                        pending.pop(0)()
                    for n in range(NCH):
                        stg = sp.tile([1, 512], F32, tag="stg")
                        nc.vector.tensor_copy(stg[:], psy[n][:])
                        nc.sync.dma_start(
                            logits[a:a + 1,
                                   hf * QTR + n * 512:hf * QTR + (n + 1) * 512],
                            stg[:])
            yt = lg.tile([APC, B], F32, tag="yt")
            nc.scalar.activation(yt[:], logits[:], AF.Sigmoid, bias=b2t[:])
            nc.sync.dma_start(y_d[:], yt[:])

        if repeat and repeat > 1:
            with tc.For_i(0, repeat, 1):
                body()
        else:
            body()
    nc.compile()
    return nc


class _Runner:
    """jit-once PJRT runner for a prebuilt Bass program (8-core SPMD)."""

    def __init__(self, nc, n_cores):
        bass2jax.install_neuronx_cc_hook()
        self.nc = nc
        self.n_cores = n_cores
        in_names, out_names, out_avals, zero_outs = [], [], [], []
        for alloc in nc.m.functions[0].allocations:
            if not isinstance(alloc, mybir.MemoryLocationSet):
                continue
            name = alloc.memorylocations[0].name
            if alloc.kind == "ExternalInput":
                in_names.append(name)
            elif alloc.kind == "ExternalOutput":
                shape = tuple(alloc.tensor_shape)
                dtype = mybir.dt.np(alloc.dtype)
                out_names.append(name)
                out_avals.append(jax.core.ShapedArray(shape, dtype))
                zero_outs.append(np.zeros(shape, dtype))
        partition_name = (nc.partition_id_tensor.name
                          if nc.partition_id_tensor else None)
        if partition_name is not None and partition_name in in_names:
            in_names.remove(partition_name)
        self.in_names = in_names
        self.out_names = out_names
        self.zero_outs = zero_outs
        n_params = len(in_names)
        n_outs = len(out_avals)
        all_in_names = list(in_names) + list(out_names)
        if partition_name is not None:
            all_in_names.append(partition_name)
        donate = tuple(range(n_params, n_params + n_outs))

        def _body(*args):
            operands = list(args)
            if partition_name is not None:
                operands.append(bass2jax.partition_id_tensor())
            outs = bass2jax._bass_exec_p.bind(
                *operands,
                out_avals=tuple(out_avals),
                in_names=tuple(all_in_names),
                out_names=tuple(out_names),
                lowering_input_output_aliases=(),
                sim_require_finite=True,
                sim_require_nnan=True,
                nc=nc,
            )
            return tuple(outs)

        devices = jax.devices()[:n_cores]
        assert len(devices) == n_cores, f"need {n_cores} neuron cores"
        self.mesh = Mesh(np.asarray(devices), ("core",))
        in_specs = (PartitionSpec("core"),) * (n_params + n_outs)
        out_specs = (PartitionSpec("core"),) * n_outs
        self.fn = jax.jit(
            shard_map(_body, mesh=self.mesh, in_specs=in_specs,
                      out_specs=out_specs, check_rep=False),
            donate_argnums=donate, keep_unused=True,
        )
        self._dev_inputs = None

    def put_inputs(self, in_maps):
        sharding = NamedSharding(self.mesh, PartitionSpec("core"))
        self._dev_inputs = [
            jax.device_put(
                np.concatenate([np.asarray(m[name]) for m in in_maps], axis=0),
                sharding)
            for name in self.in_names
        ]

    def run(self):
        sharding = NamedSharding(self.mesh, PartitionSpec("core"))
        zouts = [jax.device_put(np.concatenate([z] * self.n_cores, axis=0),
                                sharding) for z in self.zero_outs]
        outs = self.fn(*self._dev_inputs, *zouts)
        jax.block_until_ready(outs)
        return outs

    def run_np(self):
        outs = self.run()
        res = []
        for c in range(self.n_cores):
            d = {}
            for i, name in enumerate(self.out_names):
                full = np.asarray(outs[i])
                per = full.shape[0] // self.n_cores
                d[name] = full[c * per:(c + 1) * per]
            res.append(d)
        return res


_CACHE = {}


def _get_runner(repeat=0):
    if repeat not in _CACHE:
        _CACHE[repeat] = _Runner(build_program(repeat), NCORES)
    return _CACHE[repeat]


def _q8(a, scale):
    return np.clip(np.asarray(a, np.float32) * scale,
                   -240.0, 240.0).astype(E4NP)


def make_in_maps(x, W1, b1, W2, b2):
    x = np.asarray(x, dtype=np.float32)
    W1 = np.asarray(W1, dtype=np.float32)
    b1 = np.asarray(b1, dtype=np.float32)
    W2 = np.asarray(W2, dtype=np.float32)
    b2 = np.asarray(b2, dtype=np.float32)
    # x8[(hf*KC+c), p, i*QTR+n] = fp8(SX * x[hf*QTR+n, c*256+i*128+p]):
    # per-chunk contiguous 512 KiB blocks matching the [128, 2, QTR] tiles
    xq = _q8(x, SX).T
    x8 = np.ascontiguousarray(
        xq.reshape(KC, 2, 128, NQ, QTR).transpose(3, 0, 2, 1, 4)
        .reshape(NQ * KC, 128, 2 * QTR))
    # W1p[a, m, p, ks, c] = fp8(SW * W1[a, ks*128+p, m*128+c]) (per-(a,m)
    # contiguous 256 KiB block; DoubleRow pairs are ks slots (2c, 2c+1))
    W1p = np.ascontiguousarray(
        _q8(W1, SW).reshape(A, KT, 128, MT, 128).transpose(0, 3, 2, 1, 4)
        .reshape(A, MT, 128, KT, 128))
    b1p = np.ascontiguousarray(b1.reshape(A, MT, 128).transpose(0, 2, 1))
    W2p = np.ascontiguousarray(
        W2.reshape(A, MT, 128).transpose(0, 2, 1).astype(BFNP))
    b2p = np.ascontiguousarray(b2.reshape(A, 1))
    in_maps = []
    for c in range(NCORES):
        s = slice(c * APC, (c + 1) * APC)
        in_maps.append({"x8": x8, "w1p": W1p[s], "b1p": b1p[s],
                        "w2p": W2p[s], "b2p": b2p[s]})
    return in_maps


def kernel(x, W1, b1, W2, b2):
    in_maps = make_in_maps(x, W1, b1, W2, b2)
    r = _get_runner(0)
    r.put_inputs(in_maps)
    outs = r.run_np()
    y = np.concatenate([outs[c]["y"] for c in range(NCORES)], axis=0)
    return np.ascontiguousarray(y.T).astype(np.float32)
